# revision 53
# baseline (speedup 1.0000x reference)
"""Causal single-head attention (N=4096, din=dout=4096) on 8 TRN2 NeuronCores.

Math (reference):
    q = x @ Wq.T ; k = x @ Wk.T ; v = x @ Wv.T
    scores = q @ k.T ; keep j >= i (triu), else -inf
    out = softmax(scores / sqrt(N)) @ v

Reformulation (associativity moves the sharding point so each core computes
only its own 512 output rows with zero redundant FLOPs and no collectives):
    scores = (x Wq.T) Wk x.T          -> per-core: q = xr @ Wq.T ; t = q @ Wk ;
                                          scores_rows = t @ x.T
    out = (attn @ x) @ Wv.T           -> per-core: u = attn @ x ; out = u @ Wv.T

Causal-triangle work skipping with a *uniform* instruction stream across the 8
SPMD cores: core c owns 256-row chunks c (slot P) and 8+c (slot Q).  Its xT /
xw inputs are the host-shifted (by 256*c) and zero-padded transposed/natural x,
so the j-tile loop bounds are core-independent; out-of-range j-tiles read zeros
and are killed by a per-tile -30000 bias folded into the exp() activation.
Slot Q's j-tiles coincide with the second half of slot P's shifted range, so
one shifted array serves both slots.

Precision strategy (fp8 e4m3 DoubleRow matmuls at 2x PE throughput):
  * Projections (q, t, out) and score matmuls run on fp8 operands with fp32
    PSUM accumulation; per-element quantization noise averages out over the
    K=4096 contractions.  Host weights are pre-scaled by N (a power of two) so
    their fp8 encodings don't underflow; PSUM copy-out divides back exactly.
  * The value path is mean-centered: u'' = attn @ (x - 0.5) = u - 0.5, which
    keeps the fp8 u tiles bias-free (x clusters at 0.5, an e4m3 exponent
    boundary with asymmetric rounding).  Step 6 adds back the exact fp32
    0.5*rowsum(Wv) correction, which also cancels most of Wv's fp8 error.
  * Softmax weights are scale-sensitive only where the causal window is
    small.  Slot P windows are >= 2048 entries, so slot P attn can be fp8;
    slot Q windows go down to 1 entry, so slot Q attn stays bf16 and its
    attn @ x matmuls run in bf16.

Dispatch-cost note: per-dispatch overhead in this PJRT/axon path is ~30us per
kernel *parameter*, independent of bytes.  All inputs are therefore packed
into ONE fp8-typed parameter (bf16/fp32 blocks reached via AP bitcast), laid
out host-side in exact SBUF tile order so every load is a contiguous-line DMA.
"""

import sys

sys.path.insert(0, "/opt/trn_rl_repo")

from contextlib import ExitStack

import numpy as np

from concourse import bacc, mybir
from concourse.bass_utils import run_bass_kernel_spmd
from concourse.tile import TileContext

F32 = mybir.dt.float32
F8 = mybir.dt.float8e4
BF16 = mybir.dt.bfloat16
DR = mybir.MatmulPerfMode.DoubleRow
EXP = mybir.ActivationFunctionType.Exp
NEG = -30000.0
P = 128


def _layout(N):
    """Byte offsets of every block inside the single packed input.

    bf16/fp32 blocks live in the same fp8-typed tensor via AP bitcast; every
    block size is a multiple of 4 bytes so alignment holds throughout.
    """
    NT = N // P
    KP, KQ, OG = NT, NT // 2, N // 512
    SLAB = P * NT * 512

    l8, off = {}, 0

    def a8(name, nbytes):
        nonlocal off
        l8[name] = off
        off += nbytes

    a8("xr", SLAB)
    for og in range(OG):
        a8(f"wq{og}", SLAB)
    for og in range(OG):
        a8(f"wk{og}", SLAB)
    for g in range(KP // 4):
        a8(f"xc{g}", SLAB)  # [P, NT, 4P] with 4P == 512
    for g in range(NT // 4):
        a8(f"xw{g}", SLAB)  # [P, KP, 4P]
    for og in range(OG):
        a8(f"wv{og}", SLAB)
    a8("m0", P * 256)
    a8("m1", P * 256)
    a8("ones", P * 256)
    for g in range(NT // 4):
        a8(f"xwh{g}", P * KQ * 512 * 2)  # bf16
    a8("m0h", P * 256 * 2)
    a8("m1h", P * 256 * 2)
    a8("onesh", P * P * 2)
    a8("corr", P * N * 4)  # fp32
    a8("jb", P * (KP + KQ) * 4)
    return l8, off


def build_nc(N, ncores, debug=False, no_compute=False, no_dma=False):
    """One SPMD module; all per-core variation comes through the inputs.

    no_compute: keep every dma_start, strip PE/Act/DVE ops (DMA probe).
    no_dma: keep compute, strip the big input DMA loads (PE probe).
    """
    NT = N // P          # number of 128-wide tiles along any axis
    KP = NT              # slot P j-tile slots
    KQ = NT // 2         # slot Q j-tile slots
    HOFF = KP // 2       # merged-loop k at which slot Q work begins
    OG = N // 512        # 512-wide output column groups
    NP = NT // 2         # DoubleRow K-pair count along any 4096 contraction
    scale = 1.0 / float(np.sqrt(N))
    l8, sz8 = _layout(N)

    nc = bacc.Bacc("TRN2", target_bir_lowering=False)

    _skip = lambda *a, **k: None  # noqa: E731
    MM = _skip if no_compute else nc.tensor.matmul
    ACT = _skip if no_compute else nc.scalar.activation

    class _Vec:
        def __getattr__(self, n):
            return _skip if no_compute else getattr(nc.vector, n)

    VEC = _Vec()

    d_pk8 = nc.declare_dram_parameter("pk8", [sz8], F8, isOutput=False)
    d_out = nc.declare_dram_parameter("out", [512, N], F32, isOutput=True)
    if debug:
        d_dbg_q = nc.declare_dram_parameter("dbg_q", [N, 512], F32, isOutput=True)
        d_dbg_t = nc.declare_dram_parameter("dbg_t", [N, 512], F32, isOutput=True)
        d_dbg_aP = nc.declare_dram_parameter("dbg_aP", [N, 256], F32, isOutput=True)
        d_dbg_aQ = nc.declare_dram_parameter(
            "dbg_aQ", [N // 2, 256], F32, isOutput=True
        )
        d_dbg_u = nc.declare_dram_parameter("dbg_u", [N, 512], F32, isOutput=True)
        d_dbg_r = nc.declare_dram_parameter("dbg_r", [P, 512], F32, isOutput=True)

    def src(name, shape, dt=F8):
        """Packed-input slice as a [P, ...shape] access pattern of dtype dt
        (contiguous per-partition lines; host stores blocks in this order;
        bf16/fp32 blocks are reached by bitcasting the fp8 byte range)."""
        esize = {F8: 1, BF16: 2, F32: 4}[dt]
        nbytes = P * int(np.prod(shape)) * esize
        flat = d_pk8[l8[name] : l8[name] + nbytes]
        if dt != F8:
            flat = flat.bitcast(dt)
        if len(shape) == 1:
            return flat.rearrange("(p f) -> p f", p=P)
        return flat.rearrange("(p t f) -> p t f", p=P, t=shape[0])

    def _touch(ap):
        # no_dma probe: token write so read-only tiles count as allocated
        if no_dma:
            nc.vector.memset(ap, 0)

    def dma_in(engine, out, name, shape, dt=F8):
        if not no_dma:
            engine(out=out, in_=src(name, shape, dt))

    with nc.allow_low_precision(reason="fp8/bf16 tiles; fp32 PSUM accumulation throughout"), TileContext(nc) as tc:
        with ExitStack() as ctx:
            # Const tiles; their loads are emitted after xr (scalar queue)
            # and after step 1 (corr, sync queue) so the first weight slab is
            # never queued behind them -- they aren't needed before step 3
            # (corr not before step 6).
            const = ctx.enter_context(tc.tile_pool(name="const", bufs=1))
            ones_t = const.tile([P, 2, P], F8)
            onesh_t = const.tile([P, P], BF16)
            jb_t = const.tile([P, KP + KQ], F32)
            m0_t = const.tile([P, 256], F8)
            m1_t = const.tile([P, 256], F8)
            m0h_t = const.tile([P, 256], BF16)
            m1h_t = const.tile([P, 256], BF16)
            corr_t = const.tile([P, N], F32)

            # --- step 1: qT[o, i] = (xr @ Wq.T).T ------------------------
            cm_xr = tc.tile_pool(name="xr", bufs=1, side="left")
            p_xr = cm_xr.__enter__()
            xr_t = p_xr.tile([P, NT, 512], F8)
            dma_in(nc.scalar.dma_start, xr_t[:], "xr", (NT, 512))
            _touch(xr_t[:, 0, 0:16])
            nc.scalar.dma_start(out=ones_t[:], in_=src("ones", (2, P)))
            nc.scalar.dma_start(out=onesh_t[:], in_=src("onesh", (P,), BF16))
            nc.scalar.dma_start(out=jb_t[:], in_=src("jb", (KP + KQ,), F32))
            nc.scalar.dma_start(out=m0_t[:], in_=src("m0", (256,)))
            nc.scalar.dma_start(out=m1_t[:], in_=src("m1", (256,)))
            nc.scalar.dma_start(out=m0h_t[:], in_=src("m0h", (256,), BF16))
            nc.scalar.dma_start(out=m1h_t[:], in_=src("m1h", (256,), BF16))
            cm_q = tc.tile_pool(name="qT", bufs=1, side="right")
            p_q = cm_q.__enter__()
            qT_t = p_q.tile([P, NT, 512], F8)

            def dbg_dump(tile_t, nt, f, d_dst):
                with tc.tile_pool(name="dbg", bufs=2) as p_dbg:
                    for dt in range(nt):
                        db = p_dbg.tile([P, f], F32, tag="db")
                        nc.vector.tensor_copy(out=db[:], in_=tile_t[:, dt, :])
                        nc.sync.dma_start(
                            out=d_dst[P * dt : P * (dt + 1), :], in_=db[:]
                        )

            def proj_step(wname, other_t, out_t, p_w, p_ps, copy_cb=None,
                          slab_stationary=True):
                """out_t = (slab contraction vs resident K-pairs) / N.

                Steps 1, 2, 6 share this shape: per 512-wide column group,
                one contiguous [P, NT, 512] slab load, NP DoubleRow K-pairs
                accumulating into four [P, 512] PSUM banks, matmuls
                round-robin across the banks (c4 inner) so no bank sees
                back-to-back accumulation.  One accumulation group per 2KB
                bank: opened by its first matmul, closed by its last.

                slab_stationary=True (steps 1, 2): PSUM = [slab-col-chunk,
                other-free]; False (step 6): PSUM = [other-col-chunk,
                slab-free] -- i.e. [i, o] with the Wv slab moving.
                """
                for og in range(OG):
                    ws = p_w.tile([P, NT, 512], F8, tag="ws")
                    dma_in(nc.sync.dma_start, ws[:], f"{wname}{og}", (NT, 512))
                    _touch(ws[:, 0, 0:16])
                    stat_t, mov_t = (ws, other_t) if slab_stationary else (other_t, ws)
                    pss = [
                        p_ps.tile([P, 512], F32, tag="ps", name=f"ps_{og}_{i}")
                        for i in range(4)
                    ]
                    for kp in range(NP):
                        for ih in range(2):
                            for c4 in range(4):
                                MM(
                                    pss[c4][:, 256 * ih : 256 * ih + 256],
                                    lhsT=stat_t[
                                        :, 2 * kp : 2 * kp + 2, P * c4 : P * (c4 + 1)
                                    ],
                                    rhs=mov_t[
                                        :, 2 * kp : 2 * kp + 2, 256 * ih : 256 * ih + 256
                                    ],
                                    start=(kp == 0 and ih == 0),
                                    stop=(kp == NP - 1 and ih == 1),
                                    perf_mode=DR,
                                )
                    for c4 in range(4):
                        if copy_cb is None:
                            VEC.tensor_scalar_mul(
                                out_t[:, 4 * og + c4, :], pss[c4][:], 1.0 / N
                            )
                        else:
                            copy_cb(og, c4, pss[c4])

            with tc.tile_pool(name="wq_s", bufs=3) as p_wq, tc.tile_pool(
                name="ps1", bufs=8, space="PSUM"
            ) as p_ps1:
                proj_step("wq", xr_t, qT_t, p_wq, p_ps1)
            # corr load rides the sync queue here, behind step-1's slabs and
            # ahead of step-2's -- absorbed by wk prefetch slack.
            nc.sync.dma_start(out=corr_t[:], in_=src("corr", (N,), F32))
            if debug:
                dbg_dump(qT_t, NT, 512, d_dbg_q)
            cm_xr.__exit__(None, None, None)

            # --- step 2: tT[d', i] = (q @ Wk).T --------------------------
            cm_t = tc.tile_pool(name="tT", bufs=1, side="left")
            p_t = cm_t.__enter__()
            tT_t = p_t.tile([P, NT, 512], F8)
            cm_xc = tc.tile_pool(name="xc", bufs=3, side="left")
            p_xc = cm_xc.__enter__()

            with tc.tile_pool(name="wk_s", bufs=3) as p_wk, tc.tile_pool(
                name="ps2", bufs=8, space="PSUM"
            ) as p_ps2:
                proj_step("wk", qT_t, tT_t, p_wk, p_ps2)
            if debug:
                dbg_dump(tT_t, NT, 512, d_dbg_t)
            cm_q.__exit__(None, None, None)

            # --- step 3: scoresT[j, i] per j-tile; exp; mask; denom ------
            cm_a = tc.tile_pool(name="attn", bufs=1, side="right")
            p_a = cm_a.__enter__()
            attnP = p_a.tile([P, KP, 256], F8)
            attnQ = p_a.tile([P, KQ, 256], BF16)
            recP = p_a.tile([P, 256], F32)
            recQ = p_a.tile([P, 256], F32)
            cm_xw = tc.tile_pool(name="xwc", bufs=3, side="right")
            p_xw = cm_xw.__enter__()
            cm_xwh = tc.tile_pool(name="xwh", bufs=3, side="right")
            p_xwh = cm_xwh.__enter__()

            with tc.tile_pool(
                name="ps3", bufs=6, space="PSUM"
            ) as p_ps3, tc.tile_pool(name="psd", bufs=2, space="PSUM") as p_psd:
                def _step3_post(k, psP, wide):
                    ACT(
                        attnP[:, k, :],
                        psP[:, 0:256],
                        EXP,
                        bias=jb_t[:, k : k + 1],
                        scale=scale,
                    )
                    if k == 0:
                        VEC.tensor_mul(attnP[:, 0, :], attnP[:, 0, :], m0_t[:])
                    elif k == 1:
                        VEC.tensor_mul(attnP[:, 1, :], attnP[:, 1, :], m1_t[:])
                    if wide:
                        kq = k - HOFF
                        ACT(
                            attnQ[:, kq, :],
                            psP[:, 256:512],
                            EXP,
                            bias=jb_t[:, KP + kq : KP + kq + 1],
                            scale=scale,
                        )
                        if kq == 0:
                            VEC.tensor_mul(attnQ[:, 0, :], attnQ[:, 0, :], m0h_t[:])
                        elif kq == 1:
                            VEC.tensor_mul(attnQ[:, 1, :], attnQ[:, 1, :], m1h_t[:])

                def _step3_kpair(k0, xc, xoff0):
                    # Two j-tiles in flight, matmuls alternating between two
                    # PSUM banks (both tiles of a pair share `wide`).
                    wide = k0 >= HOFF
                    psA = p_ps3.tile([P, 512], F32, tag="ps3", name=f"ps3_{k0}")
                    psB = p_ps3.tile([P, 512], F32, tag="ps3", name=f"ps3_{k0 + 1}")
                    for kp in range(NP):
                        rhsP = tT_t[:, 2 * kp : 2 * kp + 2, 0:256]
                        lhA = xc[:, 2 * kp : 2 * kp + 2, xoff0 : xoff0 + P]
                        lhB = xc[:, 2 * kp : 2 * kp + 2, xoff0 + P : xoff0 + 2 * P]
                        MM(
                            psA[:, 0:256], lhsT=lhA, rhs=rhsP,
                            start=(kp == 0), stop=(kp == NP - 1 and not wide),
                            perf_mode=DR,
                        )
                        MM(
                            psB[:, 0:256], lhsT=lhB, rhs=rhsP,
                            start=(kp == 0), stop=(kp == NP - 1 and not wide),
                            perf_mode=DR,
                        )
                        if wide:
                            rhsQ = tT_t[:, 2 * kp : 2 * kp + 2, 256:512]
                            MM(
                                psA[:, 256:512], lhsT=lhA, rhs=rhsQ,
                                start=False, stop=(kp == NP - 1), perf_mode=DR,
                            )
                            MM(
                                psB[:, 256:512], lhsT=lhB, rhs=rhsQ,
                                start=False, stop=(kp == NP - 1), perf_mode=DR,
                            )
                    _step3_post(k0, psA, wide)
                    _step3_post(k0 + 1, psB, wide)

                for kp4 in range(0, KP, 4):
                    xc = p_xc.tile([P, NT, 4 * P], F8, tag="xc")
                    dma_in(nc.scalar.dma_start, xc[:], f"xc{kp4 // 4}", (NT, 4 * P))
                    _touch(xc[:, 0, 0:16])
                    _step3_kpair(kp4, xc, 0)
                    _step3_kpair(kp4 + 2, xc, 2 * P)

                # softmax denominators: ones-matrix matmul broadcasts the
                # column sums to every output partition; P/Q chains alternate
                # (both have NP == KQ trips).
                psdP = p_psd.tile([P, 512], F32, tag="psd")
                psdQ = p_psd.tile([P, 512], F32, tag="psd")
                for kp in range(NP):
                    MM(
                        psdP[:, 0:256],
                        lhsT=ones_t[:],
                        rhs=attnP[:, 2 * kp : 2 * kp + 2, :],
                        start=(kp == 0),
                        stop=(kp == NP - 1),
                        perf_mode=DR,
                    )
                    MM(
                        psdQ[:, 0:256],
                        lhsT=onesh_t[:],
                        rhs=attnQ[:, kp, :],
                        start=(kp == 0),
                        stop=(kp == NP - 1),
                    )
                VEC.reciprocal(recP[:], psdP[:, 0:256])
                VEC.reciprocal(recQ[:], psdQ[:, 0:256])
            if debug:
                dbg_dump(attnP, KP, 256, d_dbg_aP)
                dbg_dump(attnQ, KQ, 256, d_dbg_aQ)
                with tc.tile_pool(name="dbgr", bufs=1) as p_dbgr:
                    dbr = p_dbgr.tile([P, 512], F32)
                    nc.vector.tensor_copy(out=dbr[:, 0:256], in_=recP[:])
                    nc.vector.tensor_copy(out=dbr[:, 256:512], in_=recQ[:])
                    nc.sync.dma_start(out=d_dbg_r[:, :], in_=dbr[:])
            cm_xc.__exit__(None, None, None)
            cm_t.__exit__(None, None, None)

            # --- step 5: uT[d, i] = (attn @ x).T -------------------------
            cm_u = tc.tile_pool(name="uT", bufs=1, side="left")
            p_u = cm_u.__enter__()
            uT_t = p_u.tile([P, NT, 512], F8)

            with tc.tile_pool(
                name="ps5", bufs=6, space="PSUM"
            ) as p_ps5:
                for dp4 in range(0, NT, 4):
                    xwc = p_xw.tile([P, KP, 4 * P], F8, tag="xwc")
                    dma_in(nc.scalar.dma_start, xwc[:], f"xw{dp4 // 4}", (KP, 4 * P))
                    _touch(xwc[:, 0, 0:16])
                    xwh = p_xwh.tile([P, KQ, 4 * P], BF16, tag="xwh")
                    dma_in(
                        nc.scalar.dma_start, xwh[:], f"xwh{dp4 // 4}", (KQ, 4 * P), BF16
                    )
                    _touch(xwh[:, 0, 0:16])
                    # Two d-tiles x (P, Q) = 4 PSUM banks round-robin; P uses
                    # DoubleRow K-pairs, Q single bf16 tiles (NP == KQ trips).
                    for dt0 in range(dp4, dp4 + 4, 2):
                        x0 = P * (dt0 - dp4)
                        x1 = x0 + P
                        psuP0 = p_ps5.tile([P, 512], F32, tag="ps5")
                        psuQ0 = p_ps5.tile([P, 512], F32, tag="ps5")
                        psuP1 = p_ps5.tile([P, 512], F32, tag="ps5")
                        psuQ1 = p_ps5.tile([P, 512], F32, tag="ps5")
                        for kp in range(NP):
                            aP = attnP[:, 2 * kp : 2 * kp + 2, :]
                            aQ = attnQ[:, kp, :]
                            MM(
                                psuP0[:, 0:256],
                                lhsT=xwc[:, 2 * kp : 2 * kp + 2, x0 : x0 + P],
                                rhs=aP,
                                start=(kp == 0),
                                stop=(kp == NP - 1),
                                perf_mode=DR,
                            )
                            MM(
                                psuQ0[:, 0:256],
                                lhsT=xwh[:, kp, x0 : x0 + P],
                                rhs=aQ,
                                start=(kp == 0),
                                stop=(kp == NP - 1),
                            )
                            MM(
                                psuP1[:, 0:256],
                                lhsT=xwc[:, 2 * kp : 2 * kp + 2, x1 : x1 + P],
                                rhs=aP,
                                start=(kp == 0),
                                stop=(kp == NP - 1),
                                perf_mode=DR,
                            )
                            MM(
                                psuQ1[:, 0:256],
                                lhsT=xwh[:, kp, x1 : x1 + P],
                                rhs=aQ,
                                start=(kp == 0),
                                stop=(kp == NP - 1),
                            )
                        VEC.tensor_mul(uT_t[:, dt0, 0:256], psuP0[:, 0:256], recP[:])
                        VEC.tensor_mul(uT_t[:, dt0, 256:512], psuQ0[:, 0:256], recQ[:])
                        VEC.tensor_mul(uT_t[:, dt0 + 1, 0:256], psuP1[:, 0:256], recP[:])
                        VEC.tensor_mul(
                            uT_t[:, dt0 + 1, 256:512], psuQ1[:, 0:256], recQ[:]
                        )
            if debug:
                dbg_dump(uT_t, NT, 512, d_dbg_u)
            cm_xwh.__exit__(None, None, None)
            cm_xw.__exit__(None, None, None)
            cm_a.__exit__(None, None, None)

            # --- step 6: out[i, o] = u @ Wv.T ----------------------------
            def ob_copy(p_ob):
                def cb(og, it, ps):
                    ob = p_ob.tile([P, 512], F32, tag="ob")
                    if no_compute:
                        nc.vector.memset(ob[:], 0)
                    # out = u'' @ Wv.T + 0.5*rowsum(Wv): exact fp32 correction
                    # restores the mean term of the centered value path.
                    VEC.scalar_tensor_tensor(
                        out=ob[:],
                        in0=ps[:],
                        scalar=1.0 / N,
                        in1=corr_t[:, 512 * og : 512 * (og + 1)],
                        op0=mybir.AluOpType.mult,
                        op1=mybir.AluOpType.add,
                    )
                    nc.sync.dma_start(
                        out=d_out[P * it : P * (it + 1), 512 * og : 512 * (og + 1)],
                        in_=ob[:],
                    )

                return cb

            with tc.tile_pool(name="wv_s", bufs=3) as p_wv, tc.tile_pool(
                name="ps6", bufs=8, space="PSUM"
            ) as p_ps6, tc.tile_pool(name="ob", bufs=4) as p_ob:
                proj_step(
                    "wv", uT_t, None, p_wv, p_ps6,
                    copy_cb=ob_copy(p_ob), slab_stationary=False,
                )
            cm_u.__exit__(None, None, None)
    nc.finalize()
    return nc


def _tile3(a2d, t):
    """[t*P, F] -> [P, t, F] partition-major block (ravel gives (p t f))."""
    return np.ascontiguousarray(a2d.reshape(t, P, -1).transpose(1, 0, 2))


def host_inputs(x, Wq, Wk, Wv, ncores):
    import ml_dtypes

    f8 = ml_dtypes.float8_e4m3
    bf16 = ml_dtypes.bfloat16
    N = x.shape[0]
    NT = N // P
    KP, KQ, OG = NT, NT // 2, N // 512
    pad = 256 * (ncores - 1)
    l8, sz8 = _layout(N)

    xT8 = np.zeros((N, N + pad), f8)
    xT8[:, :N] = x.T.astype(f8)
    # Value path is mean-centered: u'' = attn @ (x - 0.5) = u - 0.5.
    xc8 = (x - 0.5).astype(f8)
    xw8 = np.zeros((N + pad, N), f8)
    xw8[:N, :] = xc8
    xw16 = np.zeros((N + pad, N), bf16)
    xw16[:N, :] = (x - 0.5).astype(bf16)
    # Weights pre-scaled by N (exact power of two) so entries ~U[0,1) survive
    # fp8 e4m3's underflow threshold; the kernel divides back on copy-out.
    wqT = (Wq.T * N).astype(f8)
    wvT = (Wv.T * N).astype(f8)
    wk = (Wk * N).astype(f8)

    jj = np.arange(P)[:, None]
    ii = np.arange(256)[None, :]
    m0 = (jj >= ii).astype(f8)
    m1 = ((jj + P) >= ii).astype(f8)
    corr = np.broadcast_to(
        0.5 * Wv.astype(np.float64).sum(axis=1).astype(np.float32)[None, :], (P, N)
    )

    def put(pk, name, block):
        b = np.ascontiguousarray(block).reshape(-1).view(f8)
        pk[l8[name] : l8[name] + b.size] = b

    # blocks identical on every core
    pk8_shared = np.zeros(sz8, f8)
    for og in range(OG):
        put(pk8_shared, f"wq{og}", _tile3(wqT[:, 512 * og : 512 * (og + 1)], NT))
        put(pk8_shared, f"wk{og}", _tile3(wk[:, 512 * og : 512 * (og + 1)], NT))
        put(pk8_shared, f"wv{og}", _tile3(wvT[:, 512 * og : 512 * (og + 1)], NT))
    put(pk8_shared, "m0", m0)
    put(pk8_shared, "m1", m1)
    put(pk8_shared, "ones", np.ones((P, 256), f8))
    put(pk8_shared, "m0h", m0.astype(bf16))
    put(pk8_shared, "m1h", m1.astype(bf16))
    put(pk8_shared, "onesh", np.ones((P, P), bf16))
    put(pk8_shared, "corr", np.ascontiguousarray(corr, np.float32))

    in_maps = []
    for c in range(ncores):
        s = 256 * c
        xTs = xT8[:, s : s + N]
        xws = xw8[s : s + N, :]
        xwhs = xw16[s + N // 2 : s + N, :]
        pk8 = pk8_shared.copy()
        xrb = np.empty((P, NT, 512), f8)
        A = xTs.reshape(NT, P, N + 0)
        xrb[:, :, 0:256] = A[:, :, 0:256].transpose(1, 0, 2)
        xrb[:, :, 256:512] = A[:, :, N // 2 : N // 2 + 256].transpose(1, 0, 2)
        put(pk8, "xr", xrb)
        for g in range(KP // 4):
            put(pk8, f"xc{g}", _tile3(xTs[:, 512 * g : 512 * (g + 1)], NT))
        for g in range(NT // 4):
            put(pk8, f"xw{g}", _tile3(xws[:, 512 * g : 512 * (g + 1)], KP))
        for g in range(NT // 4):
            put(pk8, f"xwh{g}", _tile3(xwhs[:, 512 * g : 512 * (g + 1)], KQ))
        jb = np.zeros((P, KP + KQ), np.float32)
        jb[:, KP - 2 * c : KP] = NEG
        jb[:, KP + KQ - 2 * c :] = NEG
        put(pk8, "jb", jb)
        in_maps.append({"pk8": pk8})
    return in_maps


def gather_out(results, N, ncores):
    out = np.empty((N, N), np.float32)
    for c in range(ncores):
        s = 256 * c
        out[s : s + 256] = results[c]["out"][:256]
        out[N // 2 + s : N // 2 + s + 256] = results[c]["out"][256:]
    return out


_NC_CACHE = {}


def run(x, Wq, Wk, Wv, ncores=None, trace=False, **spmd_kwargs):
    x = np.ascontiguousarray(np.asarray(x, dtype=np.float32))
    Wq = np.asarray(Wq, dtype=np.float32)
    Wk = np.asarray(Wk, dtype=np.float32)
    Wv = np.asarray(Wv, dtype=np.float32)
    N = x.shape[0]
    if ncores is None:
        ncores = N // 512
    key = (N, ncores)
    if key not in _NC_CACHE:
        _NC_CACHE[key] = build_nc(N, ncores)
    nc = _NC_CACHE[key]
    in_maps = host_inputs(x, Wq, Wk, Wv, ncores)
    br = run_bass_kernel_spmd(
        nc, in_maps, list(range(ncores)), trace=trace, **spmd_kwargs
    )
    return gather_out(br.results, N, ncores), br


def kernel(x, Wq, Wk, Wv):
    out, _ = run(x, Wq, Wk, Wv)
    return out


# revision 56
# speedup vs baseline: 3.8209x; 3.8209x over previous
"""Causal single-head attention (N=4096, din=dout=4096) on 8 TRN2 NeuronCores.

Math (reference):
    q = x @ Wq.T ; k = x @ Wk.T ; v = x @ Wv.T
    scores = q @ k.T ; keep j >= i (triu), else -inf
    out = softmax(scores / sqrt(N)) @ v

Reformulation (associativity moves the sharding point so each core computes
only its own 512 output rows with zero redundant FLOPs and no collectives):
    scores = (x Wq.T) Wk x.T          -> per-core: q = xr @ Wq.T ; t = q @ Wk ;
                                          scores_rows = t @ x.T
    out = (attn @ x) @ Wv.T           -> per-core: u = attn @ x ; out = u @ Wv.T

Causal-triangle work skipping with a *uniform* instruction stream across the 8
SPMD cores: core c owns 256-row chunks c (slot P) and 8+c (slot Q).  Its xT /
xw inputs are the host-shifted (by 256*c) and zero-padded transposed/natural x,
so the j-tile loop bounds are core-independent; out-of-range j-tiles read zeros
and are killed by a per-tile -30000 bias folded into the exp() activation.
Slot Q's j-tiles coincide with the second half of slot P's shifted range, so
one shifted array serves both slots.

Precision strategy (fp8 e4m3 DoubleRow matmuls at 2x PE throughput):
  * Projections (q, t, out) and score matmuls run on fp8 operands with fp32
    PSUM accumulation; per-element quantization noise averages out over the
    K=4096 contractions.  Host weights are pre-scaled by N (a power of two) so
    their fp8 encodings don't underflow; PSUM copy-out divides back exactly.
  * The value path is mean-centered: u'' = attn @ (x - 0.5) = u - 0.5, which
    keeps the fp8 u tiles bias-free (x clusters at 0.5, an e4m3 exponent
    boundary with asymmetric rounding).  Step 6 adds back the exact fp32
    0.5*rowsum(Wv) correction, which also cancels most of Wv's fp8 error.
  * Softmax weights are scale-sensitive only where the causal window is
    small.  Slot P windows are >= 2048 entries, so slot P attn can be fp8;
    slot Q windows go down to 1 entry, so slot Q attn stays bf16 and its
    attn @ x matmuls run in bf16.

Dispatch-cost note: per-dispatch overhead in this PJRT/axon path is ~30us per
kernel *parameter*, independent of bytes.  All inputs are therefore packed
into ONE fp8-typed parameter (bf16/fp32 blocks reached via AP bitcast), laid
out host-side in exact SBUF tile order so every load is a contiguous-line DMA.
"""

import sys

sys.path.insert(0, "/opt/trn_rl_repo")

from contextlib import ExitStack

import numpy as np

from concourse import bacc, mybir
from concourse.bass_utils import run_bass_kernel_spmd
from concourse.tile import TileContext

F32 = mybir.dt.float32
F8 = mybir.dt.float8e4
BF16 = mybir.dt.bfloat16
DR = mybir.MatmulPerfMode.DoubleRow
EXP = mybir.ActivationFunctionType.Exp
NEG = -30000.0
P = 128


def _layout(N):
    """Byte offsets of every block inside the single packed input.

    bf16/fp32 blocks live in the same fp8-typed tensor via AP bitcast; every
    block size is a multiple of 4 bytes so alignment holds throughout.
    """
    NT = N // P
    KP, KQ, OG = NT, NT // 2, N // 512
    SLAB = P * NT * 512

    l8, off = {}, 0

    def a8(name, nbytes):
        nonlocal off
        l8[name] = off
        off += nbytes

    a8("xr", SLAB)
    for og in range(OG):
        a8(f"wq{og}", SLAB)
    for og in range(OG):
        a8(f"wk{og}", SLAB)
    for g in range(KP // 4):
        a8(f"xc{g}", SLAB)  # [P, NT, 4P] with 4P == 512
    for g in range(NT // 4):
        a8(f"xw{g}", SLAB)  # [P, KP, 4P]
    for og in range(OG):
        a8(f"wv{og}", SLAB)
    a8("m0", P * 256)
    a8("m1", P * 256)
    a8("ones", P * 256)
    a8("corr", P * N * 4)  # fp32
    a8("jb", P * (KP + KQ) * 4)
    return l8, off


def build_nc(N, ncores, debug=False, no_compute=False, no_dma=False):
    """One SPMD module; all per-core variation comes through the inputs.

    no_compute: keep every dma_start, strip PE/Act/DVE ops (DMA probe).
    no_dma: keep compute, strip the big input DMA loads (PE probe).
    """
    NT = N // P          # number of 128-wide tiles along any axis
    KP = NT              # slot P j-tile slots
    KQ = NT // 2         # slot Q j-tile slots
    HOFF = KP // 2       # merged-loop k at which slot Q work begins
    OG = N // 512        # 512-wide output column groups
    NP = NT // 2         # DoubleRow K-pair count along any 4096 contraction
    scale = 1.0 / float(np.sqrt(N))
    l8, sz8 = _layout(N)

    nc = bacc.Bacc("TRN2", target_bir_lowering=False)

    _skip = lambda *a, **k: None  # noqa: E731
    MM = _skip if no_compute else nc.tensor.matmul
    ACT = _skip if no_compute else nc.scalar.activation

    class _Vec:
        def __getattr__(self, n):
            return _skip if no_compute else getattr(nc.vector, n)

    VEC = _Vec()

    d_pk8 = nc.declare_dram_parameter("pk8", [sz8], F8, isOutput=False)
    d_out = nc.declare_dram_parameter("out", [512, N], F32, isOutput=True)
    if debug:
        d_dbg_q = nc.declare_dram_parameter("dbg_q", [N, 512], F32, isOutput=True)
        d_dbg_t = nc.declare_dram_parameter("dbg_t", [N, 512], F32, isOutput=True)
        d_dbg_aP = nc.declare_dram_parameter("dbg_aP", [N, 256], F32, isOutput=True)
        d_dbg_aQ = nc.declare_dram_parameter(
            "dbg_aQ", [N // 2, 256], F32, isOutput=True
        )
        d_dbg_u = nc.declare_dram_parameter("dbg_u", [N, 512], F32, isOutput=True)
        d_dbg_r = nc.declare_dram_parameter("dbg_r", [P, 512], F32, isOutput=True)

    def src(name, shape, dt=F8):
        """Packed-input slice as a [P, ...shape] access pattern of dtype dt
        (contiguous per-partition lines; host stores blocks in this order;
        bf16/fp32 blocks are reached by bitcasting the fp8 byte range)."""
        esize = {F8: 1, BF16: 2, F32: 4}[dt]
        nbytes = P * int(np.prod(shape)) * esize
        flat = d_pk8[l8[name] : l8[name] + nbytes]
        if dt != F8:
            flat = flat.bitcast(dt)
        if len(shape) == 1:
            return flat.rearrange("(p f) -> p f", p=P)
        return flat.rearrange("(p t f) -> p t f", p=P, t=shape[0])

    def _touch(ap):
        # no_dma probe: token write so read-only tiles count as allocated
        if no_dma:
            nc.vector.memset(ap, 0)

    def dma_in(engine, out, name, shape, dt=F8):
        if not no_dma:
            engine(out=out, in_=src(name, shape, dt))

    with nc.allow_low_precision(reason="fp8/bf16 tiles; fp32 PSUM accumulation throughout"), TileContext(nc) as tc:
        with ExitStack() as ctx:
            # Const tiles; their loads are emitted after xr (scalar queue)
            # and after step 1 (corr, sync queue) so the first weight slab is
            # never queued behind them -- they aren't needed before step 3
            # (corr not before step 6).
            const = ctx.enter_context(tc.tile_pool(name="const", bufs=1))
            ones_t = const.tile([P, 2, P], F8)
            jb_t = const.tile([P, KP + KQ], F32)
            m0_t = const.tile([P, 256], F8)
            m1_t = const.tile([P, 256], F8)
            corr_t = const.tile([P, N], F32)

            # --- step 1: qT[o, i] = (xr @ Wq.T).T ------------------------
            cm_xr = tc.tile_pool(name="xr", bufs=1, side="left")
            p_xr = cm_xr.__enter__()
            xr_t = p_xr.tile([P, NT, 512], F8)
            dma_in(nc.scalar.dma_start, xr_t[:], "xr", (NT, 512))
            _touch(xr_t[:, 0, 0:16])
            nc.scalar.dma_start(out=ones_t[:], in_=src("ones", (2, P)))
            nc.scalar.dma_start(out=jb_t[:], in_=src("jb", (KP + KQ,), F32))
            nc.scalar.dma_start(out=m0_t[:], in_=src("m0", (256,)))
            nc.scalar.dma_start(out=m1_t[:], in_=src("m1", (256,)))
            cm_q = tc.tile_pool(name="qT", bufs=1, side="right")
            p_q = cm_q.__enter__()
            qT_t = p_q.tile([P, NT, 512], F8)

            def dbg_dump(tile_t, nt, f, d_dst):
                with tc.tile_pool(name="dbg", bufs=2) as p_dbg:
                    for dt in range(nt):
                        db = p_dbg.tile([P, f], F32, tag="db")
                        nc.vector.tensor_copy(out=db[:], in_=tile_t[:, dt, :])
                        nc.sync.dma_start(
                            out=d_dst[P * dt : P * (dt + 1), :], in_=db[:]
                        )

            def proj_step(wname, other_t, out_t, p_w, p_ps, copy_cb=None,
                          slab_stationary=True):
                """out_t = (slab contraction vs resident K-pairs) / N.

                Steps 1, 2, 6 share this shape: per 512-wide column group,
                one contiguous [P, NT, 512] slab load, NP DoubleRow K-pairs
                accumulating into four [P, 512] PSUM banks, matmuls
                round-robin across the banks (c4 inner) so no bank sees
                back-to-back accumulation.  One accumulation group per 2KB
                bank: opened by its first matmul, closed by its last.

                slab_stationary=True (steps 1, 2): PSUM = [slab-col-chunk,
                other-free]; False (step 6): PSUM = [other-col-chunk,
                slab-free] -- i.e. [i, o] with the Wv slab moving.
                """
                for og in range(OG):
                    ws = p_w.tile([P, NT, 512], F8, tag="ws")
                    dma_in(nc.sync.dma_start, ws[:], f"{wname}{og}", (NT, 512))
                    _touch(ws[:, 0, 0:16])
                    stat_t, mov_t = (ws, other_t) if slab_stationary else (other_t, ws)
                    pss = [
                        p_ps.tile([P, 512], F32, tag="ps", name=f"ps_{og}_{i}")
                        for i in range(4)
                    ]
                    for kp in range(NP):
                        for ih in range(2):
                            for c4 in range(4):
                                MM(
                                    pss[c4][:, 256 * ih : 256 * ih + 256],
                                    lhsT=stat_t[
                                        :, 2 * kp : 2 * kp + 2, P * c4 : P * (c4 + 1)
                                    ],
                                    rhs=mov_t[
                                        :, 2 * kp : 2 * kp + 2, 256 * ih : 256 * ih + 256
                                    ],
                                    start=(kp == 0 and ih == 0),
                                    stop=(kp == NP - 1 and ih == 1),
                                    perf_mode=DR,
                                )
                    for c4 in range(4):
                        if copy_cb is None:
                            VEC.tensor_scalar_mul(
                                out_t[:, 4 * og + c4, :], pss[c4][:], 1.0 / N
                            )
                        else:
                            copy_cb(og, c4, pss[c4])

            with tc.tile_pool(name="wq_s", bufs=3) as p_wq, tc.tile_pool(
                name="ps1", bufs=8, space="PSUM"
            ) as p_ps1:
                proj_step("wq", xr_t, qT_t, p_wq, p_ps1)
            # corr load rides the sync queue here, behind step-1's slabs and
            # ahead of step-2's -- absorbed by wk prefetch slack.
            nc.sync.dma_start(out=corr_t[:], in_=src("corr", (N,), F32))
            if debug:
                dbg_dump(qT_t, NT, 512, d_dbg_q)
            cm_xr.__exit__(None, None, None)

            # --- step 2: tT[d', i] = (q @ Wk).T --------------------------
            cm_t = tc.tile_pool(name="tT", bufs=1, side="left")
            p_t = cm_t.__enter__()
            tT_t = p_t.tile([P, NT, 512], F8)
            cm_xc = tc.tile_pool(name="xc", bufs=3, side="left")
            p_xc = cm_xc.__enter__()

            with tc.tile_pool(name="wk_s", bufs=3) as p_wk, tc.tile_pool(
                name="ps2", bufs=8, space="PSUM"
            ) as p_ps2:
                proj_step("wk", qT_t, tT_t, p_wk, p_ps2)
            if debug:
                dbg_dump(tT_t, NT, 512, d_dbg_t)
            cm_q.__exit__(None, None, None)

            # --- step 3: scoresT[j, i] per j-tile; exp; mask; denom ------
            cm_a = tc.tile_pool(name="attn", bufs=1, side="right")
            p_a = cm_a.__enter__()
            attnP = p_a.tile([P, KP, 256], F8)
            attnQ = p_a.tile([P, KQ, 256], F8)
            recP = p_a.tile([P, 256], F32)
            recQ = p_a.tile([P, 256], F32)
            cm_xw = tc.tile_pool(name="xwc", bufs=3, side="right")
            p_xw = cm_xw.__enter__()

            with tc.tile_pool(
                name="ps3", bufs=6, space="PSUM"
            ) as p_ps3, tc.tile_pool(name="psd", bufs=2, space="PSUM") as p_psd:
                def _step3_post(k, psP, wide):
                    ACT(
                        attnP[:, k, :],
                        psP[:, 0:256],
                        EXP,
                        bias=jb_t[:, k : k + 1],
                        scale=scale,
                    )
                    if k == 0:
                        VEC.tensor_mul(attnP[:, 0, :], attnP[:, 0, :], m0_t[:])
                    elif k == 1:
                        VEC.tensor_mul(attnP[:, 1, :], attnP[:, 1, :], m1_t[:])
                    if wide:
                        kq = k - HOFF
                        ACT(
                            attnQ[:, kq, :],
                            psP[:, 256:512],
                            EXP,
                            bias=jb_t[:, KP + kq : KP + kq + 1],
                            scale=scale,
                        )
                        if kq == 0:
                            VEC.tensor_mul(attnQ[:, 0, :], attnQ[:, 0, :], m0_t[:])
                        elif kq == 1:
                            VEC.tensor_mul(attnQ[:, 1, :], attnQ[:, 1, :], m1_t[:])

                def _step3_kpair(k0, xc, xoff0):
                    # Two j-tiles in flight, matmuls alternating between two
                    # PSUM banks (both tiles of a pair share `wide`).
                    wide = k0 >= HOFF
                    psA = p_ps3.tile([P, 512], F32, tag="ps3", name=f"ps3_{k0}")
                    psB = p_ps3.tile([P, 512], F32, tag="ps3", name=f"ps3_{k0 + 1}")
                    for kp in range(NP):
                        rhsP = tT_t[:, 2 * kp : 2 * kp + 2, 0:256]
                        lhA = xc[:, 2 * kp : 2 * kp + 2, xoff0 : xoff0 + P]
                        lhB = xc[:, 2 * kp : 2 * kp + 2, xoff0 + P : xoff0 + 2 * P]
                        MM(
                            psA[:, 0:256], lhsT=lhA, rhs=rhsP,
                            start=(kp == 0), stop=(kp == NP - 1 and not wide),
                            perf_mode=DR,
                        )
                        MM(
                            psB[:, 0:256], lhsT=lhB, rhs=rhsP,
                            start=(kp == 0), stop=(kp == NP - 1 and not wide),
                            perf_mode=DR,
                        )
                        if wide:
                            rhsQ = tT_t[:, 2 * kp : 2 * kp + 2, 256:512]
                            MM(
                                psA[:, 256:512], lhsT=lhA, rhs=rhsQ,
                                start=False, stop=(kp == NP - 1), perf_mode=DR,
                            )
                            MM(
                                psB[:, 256:512], lhsT=lhB, rhs=rhsQ,
                                start=False, stop=(kp == NP - 1), perf_mode=DR,
                            )
                    _step3_post(k0, psA, wide)
                    _step3_post(k0 + 1, psB, wide)

                for kp4 in range(0, KP, 4):
                    xc = p_xc.tile([P, NT, 4 * P], F8, tag="xc")
                    dma_in(nc.scalar.dma_start, xc[:], f"xc{kp4 // 4}", (NT, 4 * P))
                    _touch(xc[:, 0, 0:16])
                    _step3_kpair(kp4, xc, 0)
                    _step3_kpair(kp4 + 2, xc, 2 * P)

                # softmax denominators: ones-matrix matmul broadcasts the
                # column sums to every output partition; P/Q chains alternate
                # (Q has KQ//2 DoubleRow pairs vs P's NP).
                NPQ = KQ // 2
                psdP = p_psd.tile([P, 512], F32, tag="psd")
                psdQ = p_psd.tile([P, 512], F32, tag="psd")
                for kp in range(NP):
                    MM(
                        psdP[:, 0:256],
                        lhsT=ones_t[:],
                        rhs=attnP[:, 2 * kp : 2 * kp + 2, :],
                        start=(kp == 0),
                        stop=(kp == NP - 1),
                        perf_mode=DR,
                    )
                    if kp < NPQ:
                        MM(
                            psdQ[:, 0:256],
                            lhsT=ones_t[:],
                            rhs=attnQ[:, 2 * kp : 2 * kp + 2, :],
                            start=(kp == 0),
                            stop=(kp == NPQ - 1),
                            perf_mode=DR,
                        )
                VEC.reciprocal(recP[:], psdP[:, 0:256])
                VEC.reciprocal(recQ[:], psdQ[:, 0:256])
            if debug:
                dbg_dump(attnP, KP, 256, d_dbg_aP)
                dbg_dump(attnQ, KQ, 256, d_dbg_aQ)
                with tc.tile_pool(name="dbgr", bufs=1) as p_dbgr:
                    dbr = p_dbgr.tile([P, 512], F32)
                    nc.vector.tensor_copy(out=dbr[:, 0:256], in_=recP[:])
                    nc.vector.tensor_copy(out=dbr[:, 256:512], in_=recQ[:])
                    nc.sync.dma_start(out=d_dbg_r[:, :], in_=dbr[:])
            cm_xc.__exit__(None, None, None)
            cm_t.__exit__(None, None, None)

            # --- step 5: uT[d, i] = (attn @ x).T -------------------------
            cm_u = tc.tile_pool(name="uT", bufs=1, side="left")
            p_u = cm_u.__enter__()
            uT_t = p_u.tile([P, NT, 512], F8)

            with tc.tile_pool(
                name="ps5", bufs=6, space="PSUM"
            ) as p_ps5:
                for dp4 in range(0, NT, 4):
                    xwc = p_xw.tile([P, KP, 4 * P], F8, tag="xwc")
                    dma_in(nc.scalar.dma_start, xwc[:], f"xw{dp4 // 4}", (KP, 4 * P))
                    _touch(xwc[:, 0, 0:16])
                    # Two d-tiles x (P, Q) = 4 PSUM banks round-robin, all
                    # DoubleRow fp8.  Slot Q contracts xw j-tiles HOFF..KP
                    # (its window is the second half of the shifted range), so
                    # it shares the same xwc slabs with NPQ = KQ//2 pairs.
                    NPQ = KQ // 2
                    for dt0 in range(dp4, dp4 + 4, 2):
                        x0 = P * (dt0 - dp4)
                        x1 = x0 + P
                        psuP0 = p_ps5.tile([P, 512], F32, tag="ps5")
                        psuQ0 = p_ps5.tile([P, 512], F32, tag="ps5")
                        psuP1 = p_ps5.tile([P, 512], F32, tag="ps5")
                        psuQ1 = p_ps5.tile([P, 512], F32, tag="ps5")
                        for kp in range(NP):
                            aP = attnP[:, 2 * kp : 2 * kp + 2, :]
                            MM(
                                psuP0[:, 0:256],
                                lhsT=xwc[:, 2 * kp : 2 * kp + 2, x0 : x0 + P],
                                rhs=aP,
                                start=(kp == 0),
                                stop=(kp == NP - 1),
                                perf_mode=DR,
                            )
                            if kp < NPQ:
                                MM(
                                    psuQ0[:, 0:256],
                                    lhsT=xwc[
                                        :,
                                        HOFF + 2 * kp : HOFF + 2 * kp + 2,
                                        x0 : x0 + P,
                                    ],
                                    rhs=attnQ[:, 2 * kp : 2 * kp + 2, :],
                                    start=(kp == 0),
                                    stop=(kp == NPQ - 1),
                                    perf_mode=DR,
                                )
                            MM(
                                psuP1[:, 0:256],
                                lhsT=xwc[:, 2 * kp : 2 * kp + 2, x1 : x1 + P],
                                rhs=aP,
                                start=(kp == 0),
                                stop=(kp == NP - 1),
                                perf_mode=DR,
                            )
                            if kp < NPQ:
                                MM(
                                    psuQ1[:, 0:256],
                                    lhsT=xwc[
                                        :,
                                        HOFF + 2 * kp : HOFF + 2 * kp + 2,
                                        x1 : x1 + P,
                                    ],
                                    rhs=attnQ[:, 2 * kp : 2 * kp + 2, :],
                                    start=(kp == 0),
                                    stop=(kp == NPQ - 1),
                                    perf_mode=DR,
                                )
                        VEC.tensor_mul(uT_t[:, dt0, 0:256], psuP0[:, 0:256], recP[:])
                        VEC.tensor_mul(uT_t[:, dt0, 256:512], psuQ0[:, 0:256], recQ[:])
                        VEC.tensor_mul(uT_t[:, dt0 + 1, 0:256], psuP1[:, 0:256], recP[:])
                        VEC.tensor_mul(
                            uT_t[:, dt0 + 1, 256:512], psuQ1[:, 0:256], recQ[:]
                        )
            if debug:
                dbg_dump(uT_t, NT, 512, d_dbg_u)
            cm_xw.__exit__(None, None, None)
            cm_a.__exit__(None, None, None)

            # --- step 6: out[i, o] = u @ Wv.T ----------------------------
            def ob_copy(p_ob):
                def cb(og, it, ps):
                    ob = p_ob.tile([P, 512], F32, tag="ob")
                    if no_compute:
                        nc.vector.memset(ob[:], 0)
                    # out = u'' @ Wv.T + 0.5*rowsum(Wv): exact fp32 correction
                    # restores the mean term of the centered value path.
                    VEC.scalar_tensor_tensor(
                        out=ob[:],
                        in0=ps[:],
                        scalar=1.0 / N,
                        in1=corr_t[:, 512 * og : 512 * (og + 1)],
                        op0=mybir.AluOpType.mult,
                        op1=mybir.AluOpType.add,
                    )
                    nc.sync.dma_start(
                        out=d_out[P * it : P * (it + 1), 512 * og : 512 * (og + 1)],
                        in_=ob[:],
                    )

                return cb

            with tc.tile_pool(name="wv_s", bufs=3) as p_wv, tc.tile_pool(
                name="ps6", bufs=8, space="PSUM"
            ) as p_ps6, tc.tile_pool(name="ob", bufs=4) as p_ob:
                proj_step(
                    "wv", uT_t, None, p_wv, p_ps6,
                    copy_cb=ob_copy(p_ob), slab_stationary=False,
                )
            cm_u.__exit__(None, None, None)
    nc.finalize()
    return nc


def _tile3(a2d, t):
    """[t*P, F] -> [P, t, F] partition-major block (ravel gives (p t f))."""
    return np.ascontiguousarray(a2d.reshape(t, P, -1).transpose(1, 0, 2))


def host_inputs(x, Wq, Wk, Wv, ncores):
    import ml_dtypes

    f8 = ml_dtypes.float8_e4m3
    bf16 = ml_dtypes.bfloat16
    N = x.shape[0]
    NT = N // P
    KP, KQ, OG = NT, NT // 2, N // 512
    pad = 256 * (ncores - 1)
    l8, sz8 = _layout(N)

    xT8 = np.zeros((N, N + pad), f8)
    xT8[:, :N] = x.T.astype(f8)
    # Value path is mean-centered: u'' = attn @ (x - 0.5) = u - 0.5.
    xc8 = (x - 0.5).astype(f8)
    xw8 = np.zeros((N + pad, N), f8)
    xw8[:N, :] = xc8
    # Weights pre-scaled by N (exact power of two) so entries ~U[0,1) survive
    # fp8 e4m3's underflow threshold; the kernel divides back on copy-out.
    wqT = (Wq.T * N).astype(f8)
    wvT = (Wv.T * N).astype(f8)
    wk = (Wk * N).astype(f8)

    jj = np.arange(P)[:, None]
    ii = np.arange(256)[None, :]
    m0 = (jj >= ii).astype(f8)
    m1 = ((jj + P) >= ii).astype(f8)
    corr = np.broadcast_to(
        0.5 * Wv.astype(np.float64).sum(axis=1).astype(np.float32)[None, :], (P, N)
    )

    def put(pk, name, block):
        b = np.ascontiguousarray(block).reshape(-1).view(f8)
        pk[l8[name] : l8[name] + b.size] = b

    # blocks identical on every core
    pk8_shared = np.zeros(sz8, f8)
    for og in range(OG):
        put(pk8_shared, f"wq{og}", _tile3(wqT[:, 512 * og : 512 * (og + 1)], NT))
        put(pk8_shared, f"wk{og}", _tile3(wk[:, 512 * og : 512 * (og + 1)], NT))
        put(pk8_shared, f"wv{og}", _tile3(wvT[:, 512 * og : 512 * (og + 1)], NT))
    put(pk8_shared, "m0", m0)
    put(pk8_shared, "m1", m1)
    put(pk8_shared, "ones", np.ones((P, 256), f8))
    put(pk8_shared, "corr", np.ascontiguousarray(corr, np.float32))

    in_maps = []
    for c in range(ncores):
        s = 256 * c
        xTs = xT8[:, s : s + N]
        xws = xw8[s : s + N, :]
        pk8 = pk8_shared.copy()
        xrb = np.empty((P, NT, 512), f8)
        A = xTs.reshape(NT, P, N + 0)
        xrb[:, :, 0:256] = A[:, :, 0:256].transpose(1, 0, 2)
        xrb[:, :, 256:512] = A[:, :, N // 2 : N // 2 + 256].transpose(1, 0, 2)
        put(pk8, "xr", xrb)
        for g in range(KP // 4):
            put(pk8, f"xc{g}", _tile3(xTs[:, 512 * g : 512 * (g + 1)], NT))
        for g in range(NT // 4):
            put(pk8, f"xw{g}", _tile3(xws[:, 512 * g : 512 * (g + 1)], KP))
        jb = np.zeros((P, KP + KQ), np.float32)
        jb[:, KP - 2 * c : KP] = NEG
        jb[:, KP + KQ - 2 * c :] = NEG
        put(pk8, "jb", jb)
        in_maps.append({"pk8": pk8})
    return in_maps


def gather_out(results, N, ncores):
    out = np.empty((N, N), np.float32)
    for c in range(ncores):
        s = 256 * c
        out[s : s + 256] = results[c]["out"][:256]
        out[N // 2 + s : N // 2 + s + 256] = results[c]["out"][256:]
    return out


_NC_CACHE = {}


def run(x, Wq, Wk, Wv, ncores=None, trace=False, **spmd_kwargs):
    x = np.ascontiguousarray(np.asarray(x, dtype=np.float32))
    Wq = np.asarray(Wq, dtype=np.float32)
    Wk = np.asarray(Wk, dtype=np.float32)
    Wv = np.asarray(Wv, dtype=np.float32)
    N = x.shape[0]
    if ncores is None:
        ncores = N // 512
    key = (N, ncores)
    if key not in _NC_CACHE:
        _NC_CACHE[key] = build_nc(N, ncores)
    nc = _NC_CACHE[key]
    in_maps = host_inputs(x, Wq, Wk, Wv, ncores)
    br = run_bass_kernel_spmd(
        nc, in_maps, list(range(ncores)), trace=trace, **spmd_kwargs
    )
    return gather_out(br.results, N, ncores), br


def kernel(x, Wq, Wk, Wv):
    out, _ = run(x, Wq, Wk, Wv)
    return out
